# Initial kernel scaffold
#
"""Trainium2 Bass kernel for EuclideanCodebook (eval forward).

Problem: x (16,2048,128) f32, embed (8192,128) f32.
  dist = -(||x||^2 - 2 x.e^T + ||e||^2); ind = argmax(dist); quantize = embed[ind]
Equivalent ranking: s_k = x.e_k - ||e_k||^2/2; ind = argmax_k s_k.

Sharding: data-parallel over batch. Core c takes rows [c*4096, (c+1)*4096) of
x.reshape(32768,128); codebook replicated. Inputs are laid out host-side as
xT [128, 4096] and embedT [128, 8192] (contraction dim on partitions).

Per 128-row tile:
  - 16 fp32 matmuls (exact fp32, 4 cyc/row) -> PSUM [128, 512] chunks
    (allocated as 8 double-chunks [128, 1024] spanning 2 banks).
  - 8 DVE tensor_tensor_reduce ops: out = psum + bias (bias = -||e||^2/2,
    replicated [128, 8192] on-chip), written to an SBUF stash [128, 8192];
    accum = running per-row max.
  - top-8 over the 16 chunk maxes -> m_r; max_index over the stash -> global
    argmax index (first occurrence, matching jnp.argmax tie-breaking).
  - indirect-DMA gather embed[idx] -> quantize tile; DMA out idx + quantize.
"""

import numpy as np
from contextlib import ExitStack

import concourse.bass as bass
import concourse.tile as tile
from concourse import mybir, bacc
from concourse.bass_utils import run_bass_kernel_spmd

B, T, D = 16, 2048, 128
K = 8192
N_CORES = 8
ROWS_PER_CORE = B * T // N_CORES  # 4096
P = 128
N_TILES = ROWS_PER_CORE // P      # 32
N_CHUNK = 512                     # codes per psum bank
N_DCHUNK = 1024                   # codes per TTR op (2 banks)
N_DCHUNKS = K // N_DCHUNK         # 8

_CACHED = {}


def build():
    nc = bacc.Bacc()
    f32 = mybir.dt.float32

    xT = nc.declare_dram_parameter("xT", [P, ROWS_PER_CORE], f32, isOutput=False)
    embedT = nc.declare_dram_parameter("embedT", [P, K], f32, isOutput=False)
    embed = nc.declare_dram_parameter("embed", [K, D], f32, isOutput=False)
    quant = nc.declare_dram_parameter("quant", [ROWS_PER_CORE, D], f32, isOutput=True)
    eind = nc.declare_dram_parameter("eind", [ROWS_PER_CORE, 1], mybir.dt.int32, isOutput=True)

    NEG_INF = -3.0e38

    with tile.TileContext(nc) as tc:
        with ExitStack() as ctx:
            const = ctx.enter_context(tc.tile_pool(name="const", bufs=1))
            work = ctx.enter_context(tc.tile_pool(name="work", bufs=2))
            small = ctx.enter_context(tc.tile_pool(name="small", bufs=3))
            psum = ctx.enter_context(tc.tile_pool(name="psum", bufs=4, space="PSUM"))
            psq = ctx.enter_context(tc.tile_pool(name="psq", bufs=2, space="PSUM"))

            # resident inputs
            et = const.tile([P, K], f32)          # embedT
            nc.sync.dma_start(out=et[:], in_=embedT[:])
            xt = const.tile([P, ROWS_PER_CORE], f32)  # xT
            nc.sync.dma_start(out=xt[:], in_=xT[:])

            ones_col = const.tile([P, 1], f32)    # ones [128,1] (lhsT for col-sum)
            nc.vector.memset(ones_col[:], 1.0)
            ones_row = const.tile([1, P], f32)    # ones [1,128] (lhsT for bcast)
            nc.vector.memset(ones_row[:], 1.0)

            # ---- bias = -||e||^2/2 replicated to [128, K] ----
            sq = work.tile([P, K], f32, tag="stash")   # scratch; same slot as stash
            nc.scalar.square(sq[:], et[:])
            e2row = const.tile([1, K], f32)
            for c in range(K // N_CHUNK):
                cs = slice(c * N_CHUNK, (c + 1) * N_CHUNK)
                pe2 = psq.tile([1, N_CHUNK], f32, tag="pe2")
                nc.tensor.matmul(out=pe2[:], lhsT=ones_col[:], rhs=sq[:, cs],
                                 start=True, stop=True)
                # e2row = -0.5 * sum(sq)
                nc.scalar.mul(e2row[:, cs], pe2[:], -0.5)
            bias = const.tile([P, K], f32)
            for c in range(K // N_CHUNK):
                cs = slice(c * N_CHUNK, (c + 1) * N_CHUNK)
                pb = psq.tile([P, N_CHUNK], f32, tag="pb")
                nc.tensor.matmul(out=pb[:], lhsT=ones_row[:], rhs=e2row[:, cs],
                                 start=True, stop=True)
                nc.scalar.copy(bias[:, cs], pb[:])

            ninf8 = const.tile([P, 8], f32)
            nc.vector.memset(ninf8[:], NEG_INF)

            # ---- main loop over 32 row-tiles ----
            for t in range(N_TILES):
                xt_t = xt[:, t * P:(t + 1) * P]
                stash = work.tile([P, K], f32, tag="stash")
                cmax = small.tile([P, 16], f32, tag="cmax")
                for dc in range(N_DCHUNKS):
                    pch = psum.tile([P, N_DCHUNK], f32, tag="pch")
                    for h in range(2):
                        c = dc * 2 + h
                        nc.tensor.matmul(
                            out=pch[:, h * N_CHUNK:(h + 1) * N_CHUNK],
                            lhsT=xt_t,
                            rhs=et[:, c * N_CHUNK:(c + 1) * N_CHUNK],
                            start=True, stop=True,
                        )
                    dsl = slice(dc * N_DCHUNK, (dc + 1) * N_DCHUNK)
                    nc.vector.tensor_tensor_reduce(
                        out=stash[:, dsl],
                        in0=pch[:],
                        in1=bias[:, dsl],
                        scale=1.0,
                        scalar=NEG_INF,
                        op0=mybir.AluOpType.add,
                        op1=mybir.AluOpType.max,
                        accum_out=cmax[:, dc:dc + 1],
                    )
                # top-8 of the 8 chunk maxes (pad to 16 with -inf for free size>=8)
                nc.vector.tensor_copy(cmax[:, 8:16], ninf8[:])
                top8 = small.tile([P, 8], f32, tag="top8")
                nc.vector.max(top8[:], cmax[:])
                idx8 = small.tile([P, 8], mybir.dt.uint32, tag="idx8")
                nc.vector.max_index(idx8[:], top8[:], stash[:])

                # gather quantize rows and write outputs
                qtile = small.tile([P, D], f32, tag="qtile")
                nc.gpsimd.indirect_dma_start(
                    out=qtile[:],
                    out_offset=None,
                    in_=embed[:],
                    in_offset=bass.IndirectOffsetOnAxis(
                        ap=idx8[:, :1].bitcast(mybir.dt.int32), axis=0,
                    ),
                )
                rsl = slice(t * P, (t + 1) * P)
                nc.sync.dma_start(out=quant[rsl, :], in_=qtile[:])
                nc.sync.dma_start(out=eind[rsl, :], in_=idx8[:, :1].bitcast(mybir.dt.int32))

    nc.compile()
    return nc


def kernel(x, x_len, embed):
    x = np.ascontiguousarray(np.asarray(x, dtype=np.float32))
    embed = np.ascontiguousarray(np.asarray(embed, dtype=np.float32))
    b, t, d = x.shape
    assert (b, t, d) == (B, T, D) and embed.shape == (K, D)

    if "nc" not in _CACHED:
        _CACHED["nc"] = build()
    nc = _CACHED["nc"]

    xf = x.reshape(-1, D)                       # (32768, 128)
    embedT = np.ascontiguousarray(embed.T)      # (128, 8192)

    in_maps = []
    for c in range(N_CORES):
        sh = xf[c * ROWS_PER_CORE:(c + 1) * ROWS_PER_CORE]
        in_maps.append({
            "xT": np.ascontiguousarray(sh.T),   # (128, 4096)
            "embedT": embedT,
            "embed": embed,
        })

    res = run_bass_kernel_spmd(nc, in_maps, list(range(N_CORES))).results

    quant = np.concatenate([r["quant"] for r in res], axis=0).reshape(b, t, d)
    eind = np.concatenate([r["eind"][:, 0] for r in res], axis=0).reshape(b, t)
    commit_loss = np.float32(0.0)
    return quant, commit_loss, eind.astype(np.int32)


if __name__ == "__main__":
    rng = np.random.default_rng(0)
    x = rng.standard_normal((B, T, D), dtype=np.float32)
    x_len = rng.integers(0, T, (B,), dtype=np.int32)
    emb = rng.standard_normal((K, D), dtype=np.float32)
    q, cl, ei = kernel(x, x_len, emb)
    s = xf = x.reshape(-1, D) @ emb.T - 0.5 * (emb ** 2).sum(1)[None, :]
    ref_ei = s.argmax(1)
    print("idx match:", (ei.reshape(-1) == ref_ei).mean())
    print("quant err:", np.abs(q.reshape(-1, D) - emb[ref_ei]).max())


# revision 8
# speedup vs baseline: 1.0309x; 1.0309x over previous
"""Trainium2 Bass kernel for EuclideanCodebook (eval forward).

Problem: x (16,2048,128) f32, embed (8192,128) f32.
  dist = -(||x||^2 - 2 x.e^T + ||e||^2); ind = argmax(dist); quantize = embed[ind]
Equivalent ranking: s_k = x.e_k - ||e_k||^2/2 = argmax target.

Sharding: data-parallel over batch. Core c takes rows [c*4096, (c+1)*4096) of
x.reshape(32768,128); codebook replicated. Host prep is layout/dtype only:
transpose to put the contraction dim on partitions, and split fp32 into
exact (hi, lo) fp16 pairs (x = xh + xl with |xl| <= 2^-12|x|).

Matmul: s = x.e is computed as xh.eh + xh.el + xl.eh (fp32 PSUM accumulate;
dropped xl.el term ~2^-24) -- fp32-grade precision at 3 cycles/row instead of
fp32's 4. The -||e||^2/2 bias is computed on device (square + (-0.5)-column-sum
matmul), split hi/lo fp16 via PSUM arithmetic, and folded into the same PSUM
accumulation group as a single K=2 ones-matmul.

Per 128-row tile:
  - 8 double-chunks [128,1024]: 2x(4 matmuls) -> biased scores in PSUM
  - DVE max (top8) per double-chunk; ACT copies PSUM -> SBUF stash
  - reduce_max of chunk maxes -> m_r; max_index(m_r, stash) -> argmax index
    (first occurrence, matching jnp.argmax tie-breaking)
  - indirect-DMA gather embed[idx] -> quantize tile; DMA out idx + quantize
"""

import numpy as np
from contextlib import ExitStack

import concourse.bass as bass
import concourse.tile as tile
from concourse import mybir, bacc
from concourse.bass_utils import run_bass_kernel_spmd

B, T, D = 16, 2048, 128
K = 8192
N_CORES = 8
ROWS_PER_CORE = B * T // N_CORES  # 4096
P = 128
N_TILES = ROWS_PER_CORE // P      # 32
NC = 512                          # codes per psum bank
ND = 1024                         # codes per double-chunk
NEG_INF = -3.0e38

_CACHED = {}


def build():
    nc = bacc.Bacc()
    f32 = mybir.dt.float32
    f16 = mybir.dt.float16

    xh_d = nc.declare_dram_parameter("xh", [P, ROWS_PER_CORE], f16, isOutput=False)
    xl_d = nc.declare_dram_parameter("xl", [P, ROWS_PER_CORE], f16, isOutput=False)
    eh_d = nc.declare_dram_parameter("eh", [P, K], f16, isOutput=False)
    el_d = nc.declare_dram_parameter("el", [P, K], f16, isOutput=False)
    emb_d = nc.declare_dram_parameter("emb", [K, D], f32, isOutput=False)
    quant = nc.declare_dram_parameter("quant", [ROWS_PER_CORE, D], f32, isOutput=True)
    eind = nc.declare_dram_parameter("eind", [ROWS_PER_CORE, 1], mybir.dt.int32, isOutput=True)

    with tile.TileContext(nc) as tc:
        with ExitStack() as ctx:
            const = ctx.enter_context(tc.tile_pool(name="const", bufs=1))
            work = ctx.enter_context(tc.tile_pool(name="work", bufs=2))
            small = ctx.enter_context(tc.tile_pool(name="small", bufs=3))
            psum = ctx.enter_context(tc.tile_pool(name="psum", bufs=3, space="PSUM"))
            psq = ctx.enter_context(tc.tile_pool(name="psq", bufs=2, space="PSUM"))

            eh = const.tile([P, K], f16)
            nc.sync.dma_start(out=eh[:], in_=eh_d[:])
            el = const.tile([P, K], f16)
            nc.sync.dma_start(out=el[:], in_=el_d[:])
            xh = const.tile([P, ROWS_PER_CORE], f16)
            nc.sync.dma_start(out=xh[:], in_=xh_d[:])
            xl = const.tile([P, ROWS_PER_CORE], f16)
            nc.sync.dma_start(out=xl[:], in_=xl_d[:])

            neghalf_col = const.tile([P, 1], f32)   # lhsT for -0.5 * column sum
            nc.vector.memset(neghalf_col[:], -0.5)
            negone1 = const.tile([1, 1], f16)
            nc.vector.memset(negone1[:], -1.0)
            ones2 = const.tile([2, P], f16)         # lhsT broadcasting 2 bias rows
            nc.vector.memset(ones2[:], 1.0)

            # ---- bias2 = hi/lo fp16 split of -||e||^2/2, via PSUM only ----
            # e = eh + el exactly (fp32); sq = e^2; colsum with -0.5 -> psum;
            # hi = f16(psum); psum += (-1)*hi -> lo residue; lo = f16(psum).
            ef = work.tile([P, K], f32, tag="stash")
            nc.vector.tensor_tensor(out=ef[:], in0=eh[:], in1=el[:], op=mybir.AluOpType.add)
            sq = work.tile([P, K], f32, tag="stash")
            nc.scalar.square(sq[:], ef[:])
            bias2 = const.tile([2, K], f16)
            bh_row = const.tile([1, K], f16)
            bl_row = const.tile([1, K], f16)
            for c in range(K // NC):
                cs = slice(c * NC, (c + 1) * NC)
                pe2 = psq.tile([1, NC], f32, tag="pe2")
                nc.tensor.matmul(out=pe2[:], lhsT=neghalf_col[:], rhs=sq[:, cs],
                                 start=True, stop=False)
                nc.scalar.copy(bh_row[0:1, cs], pe2[:])
                nc.tensor.matmul(out=pe2[:], lhsT=negone1[:], rhs=bh_row[0:1, cs],
                                 start=False, stop=True)
                nc.scalar.copy(bl_row[0:1, cs], pe2[:])
            # engines can't address a partition-1 start; DMA the rows into place
            nc.sync.dma_start(out=bias2[0:1, :], in_=bh_row[:])
            nc.sync.dma_start(out=bias2[1:2, :], in_=bl_row[:])

            # ---- main loop ----
            for t in range(N_TILES):
                ts = slice(t * P, (t + 1) * P)
                xh_t = xh[:, ts]
                xl_t = xl[:, ts]
                stash = work.tile([P, K], f32, tag="stash")
                for dc in range(K // ND):
                    pch = psum.tile([P, ND], f32, tag="pch")
                    for h in range(2):
                        c = dc * 2 + h
                        cs = slice(c * NC, (c + 1) * NC)
                        ps = pch[:, h * NC:(h + 1) * NC]
                        nc.tensor.matmul(out=ps, lhsT=xh_t, rhs=eh[:, cs], start=True, stop=False)
                        nc.tensor.matmul(out=ps, lhsT=xh_t, rhs=el[:, cs], start=False, stop=False)
                        nc.tensor.matmul(out=ps, lhsT=xl_t, rhs=eh[:, cs], start=False, stop=False)
                        nc.tensor.matmul(out=ps, lhsT=ones2[:], rhs=bias2[:, cs], start=False, stop=True)
                    dsl = slice(dc * ND, (dc + 1) * ND)
                    nc.scalar.copy(stash[:, dsl], pch[:])
                top8 = small.tile([P, 8], f32, tag="top8")
                nc.vector.max(top8[:], stash[:])
                idx8 = small.tile([P, 8], mybir.dt.uint32, tag="idx8")
                nc.vector.max_index(idx8[:], top8[:], stash[:])

                qtile = small.tile([P, D], f32, tag="qtile")
                nc.gpsimd.indirect_dma_start(
                    out=qtile[:], out_offset=None, in_=emb_d[:],
                    in_offset=bass.IndirectOffsetOnAxis(
                        ap=idx8[:, :1].bitcast(mybir.dt.int32), axis=0),
                )
                nc.sync.dma_start(out=quant[ts, :], in_=qtile[:])
                nc.sync.dma_start(out=eind[ts, :], in_=idx8[:, :1].bitcast(mybir.dt.int32))

    nc.compile()
    return nc


def _split16(a):
    hi = a.astype(np.float16)
    lo = (a - hi.astype(np.float32)).astype(np.float16)
    return hi, lo


def kernel(x, x_len, embed):
    x = np.ascontiguousarray(np.asarray(x, dtype=np.float32))
    embed = np.ascontiguousarray(np.asarray(embed, dtype=np.float32))
    b, t, d = x.shape
    assert (b, t, d) == (B, T, D) and embed.shape == (K, D)

    if "nc" not in _CACHED:
        _CACHED["nc"] = build()
    nc = _CACHED["nc"]

    xf = x.reshape(-1, D)
    eT = np.ascontiguousarray(embed.T)          # (128, 8192)
    eh, el = _split16(eT)

    in_maps = []
    for c in range(N_CORES):
        shT = np.ascontiguousarray(xf[c * ROWS_PER_CORE:(c + 1) * ROWS_PER_CORE].T)
        xh, xl = _split16(shT)
        in_maps.append({"xh": xh, "xl": xl, "eh": eh, "el": el, "emb": embed})

    res = run_bass_kernel_spmd(nc, in_maps, list(range(N_CORES))).results

    quant = np.concatenate([r["quant"] for r in res], axis=0).reshape(b, t, d)
    eind = np.concatenate([r["eind"][:, 0] for r in res], axis=0).reshape(b, t)
    commit_loss = np.float32(0.0)
    return quant, commit_loss, eind.astype(np.int32)


def timed_run(inputs, tmpdir=None):
    """Run once with NTFF tracing; return HW exec time in ns."""
    x = np.ascontiguousarray(np.asarray(inputs["x"], dtype=np.float32))
    embed = np.ascontiguousarray(np.asarray(inputs["embed"], dtype=np.float32))
    if "nc" not in _CACHED:
        _CACHED["nc"] = build()
    nc = _CACHED["nc"]
    xf = x.reshape(-1, D)
    eT = np.ascontiguousarray(embed.T)
    eh, el = _split16(eT)
    in_maps = []
    for c in range(N_CORES):
        shT = np.ascontiguousarray(xf[c * ROWS_PER_CORE:(c + 1) * ROWS_PER_CORE].T)
        xh, xl = _split16(shT)
        in_maps.append({"xh": xh, "xl": xl, "eh": eh, "el": el, "emb": embed})
    r = run_bass_kernel_spmd(nc, in_maps, list(range(N_CORES)), trace=True,
                             tmpdir=tmpdir)
    ns = r.exec_time_ns
    if ns is None:
        raise RuntimeError("no exec_time_ns from traced run")
    return ns


if __name__ == "__main__":
    rng = np.random.default_rng(0)
    x = rng.standard_normal((B, T, D), dtype=np.float32)
    x_len = rng.integers(0, T, (B,), dtype=np.int32)
    emb = rng.standard_normal((K, D), dtype=np.float32)
    q, cl, ei = kernel(x, x_len, emb)
    s = x.reshape(-1, D).astype(np.float64) @ emb.astype(np.float64).T \
        - 0.5 * (emb.astype(np.float64) ** 2).sum(1)[None, :]
    ref_ei = s.argmax(1)
    print("idx match:", (ei.reshape(-1) == ref_ei).mean())
    print("quant err:", np.abs(q.reshape(-1, D) - emb[ref_ei]).max())
